# revision 1
# baseline (speedup 1.0000x reference)
"""Expert-parallel MoE FFN kernel for Trainium2 (8 NeuronCores, one expert per core).

Host side: routes tokens to experts (dedup per expert, summing duplicate top-k
weights), pads each expert's token list to a common T_PAD, and pre-tiles the
weight matrices into DMA-friendly contiguous blocks.

Device side (per core, expert e):
  h^T = silu(G_e^T X^T) * (U_e^T X^T)        [I, T]   (stage A, fp32r matmuls)
  y   = (h^T)^T-contracted-with D_e * cw      [T, H]   (stage B)
All matmuls run as float32r (tf32-like rounding, 1 cycle/row on the PE vs 4
for plain fp32); accumulation is fp32 in PSUM.
"""
import sys

if "/opt/trn_rl_repo" not in sys.path:
    sys.path.insert(0, "/opt/trn_rl_repo")

import numpy as np

N_TOKENS, TOP_K, N_EXPERTS, HIDDEN, INTER = 4096, 2, 8, 1024, 2048
P = 128
NI = INTER // P          # 16 I-tiles
KH = HIDDEN // P         # 8 H(contraction)-tiles
HC = HIDDEN // 512       # 2 output-column chunks

_CACHE = {}
MM_BF16 = True


def _build(t_pad):
    import concourse.bacc as bacc
    import concourse.mybir as mybir
    import concourse.tile as tile

    f32 = mybir.dt.float32
    f32r = mybir.dt.bfloat16 if MM_BF16 else mybir.dt.float32r

    nt = t_pad // P          # T tiles of 128
    ntc = t_pad // 512       # T chunks of 512

    nc = bacc.Bacc()
    xt = nc.declare_dram_parameter("xt", [KH, P, t_pad], f32r, isOutput=False)
    gw = nc.declare_dram_parameter("gw", [NI, P, HIDDEN], f32r, isOutput=False)
    uw = nc.declare_dram_parameter("uw", [NI, P, HIDDEN], f32r, isOutput=False)
    dw = nc.declare_dram_parameter("dw", [NI, P, HIDDEN], f32r, isOutput=False)
    cw = nc.declare_dram_parameter("cw", [P, t_pad], f32, isOutput=False)
    y = nc.declare_dram_parameter("y", [HIDDEN, t_pad], f32, isOutput=True)

    with tile.TileContext(nc) as tc:
        with (
            tc.tile_pool(name="hp", bufs=1) as hp,
            tc.tile_pool(name="wp", bufs=2) as wp,
            tc.tile_pool(name="ep", bufs=3) as ep,
            tc.tile_pool(name="cp", bufs=1) as cp,
        ):
            cwt = cp.tile([P, t_pad], f32)
            nc.sync.dma_start(out=cwt[:], in_=cw[:])

            hts = [hp.tile([P, t_pad], f32r, tag=f"h{i}", name=f"ht{i}") for i in range(NI)]

            # ---- Stage A: h^T[i] = silu(G^T X^T) * (U^T X^T), tiled over I ----
            with (
                tc.tile_pool(name="xp", bufs=1) as xp,
                tc.tile_pool(name="psA", bufs=2, space="PSUM") as psA,
            ):
                xts = []
                for k in range(KH):
                    t = xp.tile([P, t_pad], f32r, tag=f"x{k}")
                    nc.sync.dma_start(out=t[:], in_=xt[k])
                    xts.append(t)
                for i in range(NI):
                    gt = wp.tile([P, HIDDEN], f32r, tag="g")
                    ut = wp.tile([P, HIDDEN], f32r, tag="u")
                    nc.sync.dma_start(out=gt[:], in_=gw[i])
                    nc.sync.dma_start(out=ut[:], in_=uw[i])
                    pgs = [psA.tile([P, 512], f32, tag=f"pg{c}", name=f"pg{i}_{c}") for c in range(ntc)]
                    pus = [psA.tile([P, 512], f32, tag=f"pu{c}", name=f"pu{i}_{c}") for c in range(ntc)]
                    for k in range(KH):
                        lg = gt[:, k * P:(k + 1) * P]
                        lu = ut[:, k * P:(k + 1) * P]
                        for c in range(ntc):
                            rx = xts[k][:, c * 512:(c + 1) * 512]
                            nc.tensor.matmul(out=pgs[c][:], lhsT=lg, rhs=rx,
                                             start=(k == 0), stop=(k == KH - 1))
                        for c in range(ntc):
                            rx = xts[k][:, c * 512:(c + 1) * 512]
                            nc.tensor.matmul(out=pus[c][:], lhsT=lu, rhs=rx,
                                             start=(k == 0), stop=(k == KH - 1))
                    for c in range(ntc):
                        sg = ep.tile([P, 512], f32, tag="sg")
                        nc.scalar.activation(out=sg[:], in_=pgs[c][:],
                                             func=mybir.ActivationFunctionType.Silu)
                        nc.vector.tensor_mul(out=hts[i][:, c * 512:(c + 1) * 512],
                                             in0=sg[:], in1=pus[c][:])

            # ---- Stage B: y^T[j,:] = sum_i D[i,j-cols]^T @ h^T[i], * cw ----
            # dw tile is the stationary operand: one weight load serves ntc
            # matmuls. Output is y^T [H, T]; host transposes back.
            jg = max(1, 8 // ntc)          # j-tiles per group, jg*ntc <= 8 banks
            with (
                tc.tile_pool(name="dwp", bufs=1) as dwp,
                tc.tile_pool(name="psB", bufs=1, space="PSUM") as psB,
            ):
                dts = []
                for i in range(NI):
                    dt_ = dwp.tile([P, HIDDEN], f32r, tag=f"d{i}", name=f"dt{i}")
                    nc.sync.dma_start(out=dt_[:], in_=dw[i])
                    dts.append(dt_)
                for j0 in range(0, KH, jg):
                    pys = [psB.tile([P, 512], f32, tag=f"py{jj}_{c}",
                                    name=f"py{j0}_{jj}_{c}")
                           for jj in range(jg) for c in range(ntc)]
                    for i in range(NI):
                        for jj in range(jg):
                            ld = dts[i][:, (j0 + jj) * P:(j0 + jj + 1) * P]
                            for c in range(ntc):
                                nc.tensor.matmul(out=pys[jj * ntc + c][:],
                                                 lhsT=ld,
                                                 rhs=hts[i][:, c * 512:(c + 1) * 512],
                                                 start=(i == 0), stop=(i == NI - 1))
                    for jj in range(jg):
                        for c in range(ntc):
                            ysb = ep.tile([P, 512], f32, tag="y")
                            nc.vector.tensor_mul(out=ysb[:],
                                                 in0=pys[jj * ntc + c][:],
                                                 in1=cwt[:, c * 512:(c + 1) * 512])
                            nc.gpsimd.dma_start(
                                out=y[(j0 + jj) * P:(j0 + jj + 1) * P,
                                      c * 512:(c + 1) * 512],
                                in_=ysb[:])

    nc.finalize()
    return nc


def _route(expert_indices, expert_weights):
    idx = np.asarray(expert_indices).astype(np.int64)
    wts = np.asarray(expert_weights).astype(np.float32)
    n = idx.shape[0]
    cw_full = np.zeros((N_EXPERTS, n), np.float32)
    for k in range(idx.shape[1]):
        np.add.at(cw_full, (idx[:, k], np.arange(n)), wts[:, k])
    ids = [np.nonzero(cw_full[e])[0] for e in range(N_EXPERTS)]
    maxc = max(len(i) for i in ids)
    t_pad = max(512, ((maxc + 511) // 512) * 512)
    return cw_full, ids, t_pad


_LDW_PATCHED = False


def _patch_ldw_opt():
    """Enable walrus's LDWEIGHTS dedup pass: consecutive matmuls that reuse the
    same stationary tile then skip the redundant ~190ns weight reload."""
    global _LDW_PATCHED
    if _LDW_PATCHED:
        return
    import concourse.bass_utils as bu

    orig = bu.run_command

    def run_command(argv, **kw):
        argv = ["--enable-ldw-opt=true" if a == "--enable-ldw-opt=false" else a
                for a in argv]
        return orig(argv, **kw)

    bu.run_command = run_command
    _LDW_PATCHED = True


def _run(nc, in_maps, trace=False, trace_cores=None):
    from concourse.bass_utils import run_bass_kernel_spmd

    return run_bass_kernel_spmd(
        nc, in_maps, list(range(N_EXPERTS)), trace=trace,
        trace_cores=trace_cores,
    )


def prepare(tokens, expert_indices, expert_weights, gate_weight, up_weight,
            down_weight):
    """Host-side routing + layout. Returns (nc, in_maps, ids, t_pad)."""
    tokens = np.ascontiguousarray(np.asarray(tokens, dtype=np.float32))
    gate_weight = np.asarray(gate_weight, dtype=np.float32)
    up_weight = np.asarray(up_weight, dtype=np.float32)
    down_weight = np.asarray(down_weight, dtype=np.float32)

    cw_full, ids, t_pad = _route(expert_indices, expert_weights)
    nt = t_pad // P

    key = t_pad
    if key not in _CACHE:
        _CACHE[key] = _build(t_pad)
    nc = _CACHE[key]

    in_maps = []
    for e in range(N_EXPERTS):
        ce = len(ids[e])
        xe = np.zeros((HIDDEN, t_pad), np.float32)
        xe[:, :ce] = tokens[ids[e]].T
        cwe = np.zeros((t_pad,), np.float32)
        cwe[:ce] = cw_full[e, ids[e]]
        mmdt = np.dtype("bfloat16") if MM_BF16 else np.float32
        in_maps.append({
            "xt": np.ascontiguousarray(xe.reshape(KH, P, t_pad)).astype(mmdt),
            "gw": np.ascontiguousarray(
                gate_weight[e].reshape(KH, P, NI, P).transpose(2, 1, 0, 3)
            ).reshape(NI, P, HIDDEN).astype(mmdt),
            "uw": np.ascontiguousarray(
                up_weight[e].reshape(KH, P, NI, P).transpose(2, 1, 0, 3)
            ).reshape(NI, P, HIDDEN).astype(mmdt),
            "dw": np.ascontiguousarray(down_weight[e].reshape(NI, P, HIDDEN)).astype(mmdt),
            "cw": np.ascontiguousarray(
                np.broadcast_to(cwe[None, :], (P, t_pad))),
        })
    return nc, in_maps, ids, t_pad


def combine(results, ids):
    out = np.zeros((N_TOKENS, HIDDEN), np.float32)
    for e in range(N_EXPERTS):
        ce = len(ids[e])
        out[ids[e]] += results[e]["y"].T[:ce]
    return out


def kernel(tokens, expert_indices, expert_weights, gate_weight, up_weight,
           down_weight):
    nc, in_maps, ids, _ = prepare(tokens, expert_indices, expert_weights,
                                  gate_weight, up_weight, down_weight)
    res = _run(nc, in_maps, trace=False)
    return combine(res.results, ids)



# revision 5
# speedup vs baseline: 1.1968x; 1.1968x over previous
"""Expert-parallel MoE FFN kernel for Trainium2 (8 NeuronCores, one expert per core).

Host side: routes tokens to experts (dedup per expert, summing duplicate top-k
weights), pads each expert's token list to a common T_PAD, and pre-tiles the
weight matrices into DMA-friendly contiguous blocks.

Device side (per core, expert e):
  h^T = silu(G_e^T X^T) * (U_e^T X^T)     [I, T]   (stage A)
  y   = (h^T)^T @ D_e  * cw               [T, H]   (stage B; h^T tiles are the
                                           stationary operand so the cw combine
                                           becomes a per-partition scale)
All matmuls are bf16 with fp32 PSUM accumulation.

Perf structure:
 - DMA issue order puts the first tiles the PE needs (x^T k=0, gate/up i=0)
   at the head of the queue, prefetches the stage-B down-weights before the
   compute-gated gate/up stream, and defers the cw vector to last.
 - One shared PSUM pool (tags q0..q3, bufs=2) spans both stages, so stage B's
   accumulators take over stage A's bank ring without a pool barrier.
 - A post-schedule pass drops LDWEIGHTS reloads whose stationary tile is
   already in the PE array.
 - Stage B drains alternate Scalar/Vector engines and the output DMAs
   alternate Sync/GpSimd queues to shorten the kernel tail.
"""
import sys

if "/opt/trn_rl_repo" not in sys.path:
    sys.path.insert(0, "/opt/trn_rl_repo")

import numpy as np

N_TOKENS, TOP_K, N_EXPERTS, HIDDEN, INTER = 4096, 2, 8, 1024, 2048
P = 128
NI = INTER // P          # 16 I-tiles
KH = HIDDEN // P         # 8 H(contraction)-tiles
NHC = HIDDEN // 512      # 2 output-column chunks

_CACHE = {}


def _build(t_pad):
    import concourse.bacc as bacc
    import concourse.mybir as mybir
    import concourse.tile as tile

    f32 = mybir.dt.float32
    bf16 = mybir.dt.bfloat16

    nt = t_pad // P          # t-blocks of 128
    ntc = t_pad // 512       # t-chunks of 512
    assert ntc == 2 and nt % 2 == 0, "V2 layout assumes t_pad multiple of 1024"

    nc = bacc.Bacc()
    xt = nc.declare_dram_parameter("xt", [KH, P, t_pad], bf16, isOutput=False)
    gw = nc.declare_dram_parameter("gw", [NI, P, HIDDEN], bf16, isOutput=False)
    uw = nc.declare_dram_parameter("uw", [NI, P, HIDDEN], bf16, isOutput=False)
    dw = nc.declare_dram_parameter("dw", [NI, P, HIDDEN], bf16, isOutput=False)
    cw = nc.declare_dram_parameter("cw", [P, nt], f32, isOutput=False)
    y = nc.declare_dram_parameter("y", [t_pad, HIDDEN], f32, isOutput=True)

    with tile.TileContext(nc) as tc:
        with (
            tc.tile_pool(name="xp", bufs=1) as xp,
            tc.tile_pool(name="hp", bufs=1) as hp,
            tc.tile_pool(name="wp", bufs=2) as wp,
            tc.tile_pool(name="dp", bufs=1) as dp,
            tc.tile_pool(name="cp", bufs=1) as cp,
            tc.tile_pool(name="ep", bufs=3) as ep,
            tc.tile_pool(name="ps", bufs=2, space="PSUM") as ps,
        ):
            # ---- DMA issue order (single Sync queue ~ HBM-saturating):
            # stage-A critical path first, stage-B weights prefetched before
            # the compute-gated gate/up stream, cw (stage-B drain only) last.
            xts = [xp.tile([P, t_pad], bf16, tag=f"x{k}") for k in range(KH)]
            nc.sync.dma_start(out=xts[0][:], in_=xt[0])
            wpre = []
            for i in range(2):
                gt = wp.tile([P, HIDDEN], bf16, tag="g", name=f"gt{i}")
                ut = wp.tile([P, HIDDEN], bf16, tag="u", name=f"ut{i}")
                nc.sync.dma_start(out=gt[:], in_=gw[i])
                nc.sync.dma_start(out=ut[:], in_=uw[i])
                wpre.append((gt, ut))
                if i == 0:
                    for k in range(1, KH):
                        nc.sync.dma_start(out=xts[k][:], in_=xt[k])
            dts = []
            for i in range(NI):
                dt_ = dp.tile([P, HIDDEN], bf16, tag=f"d{i}", name=f"dt{i}")
                nc.sync.dma_start(out=dt_[:], in_=dw[i])
                dts.append(dt_)
            cwt = cp.tile([P, nt], f32)
            nc.sync.dma_start(out=cwt[:], in_=cw[:])

            hts = [hp.tile([P, t_pad], bf16, tag=f"h{i}", name=f"ht{i}")
                   for i in range(NI)]

            # ---- Stage A: h^T[i] = silu(G^T X^T) * (U^T X^T), tiled over I ----
            for i in range(NI):
                if i < 2:
                    gt, ut = wpre[i]
                else:
                    gt = wp.tile([P, HIDDEN], bf16, tag="g", name=f"gt{i}")
                    ut = wp.tile([P, HIDDEN], bf16, tag="u", name=f"ut{i}")
                    nc.sync.dma_start(out=gt[:], in_=gw[i])
                    nc.sync.dma_start(out=ut[:], in_=uw[i])
                pgs = [ps.tile([P, 512], f32, tag=f"q{c}", name=f"pg{i}_{c}")
                       for c in range(ntc)]
                pus = [ps.tile([P, 512], f32, tag=f"q{ntc + c}", name=f"pu{i}_{c}")
                       for c in range(ntc)]
                for k in range(KH):
                    lg = gt[:, k * P:(k + 1) * P]
                    lu = ut[:, k * P:(k + 1) * P]
                    for c in range(ntc):
                        nc.tensor.matmul(out=pgs[c][:], lhsT=lg,
                                         rhs=xts[k][:, c * 512:(c + 1) * 512],
                                         start=(k == 0), stop=(k == KH - 1))
                    for c in range(ntc):
                        nc.tensor.matmul(out=pus[c][:], lhsT=lu,
                                         rhs=xts[k][:, c * 512:(c + 1) * 512],
                                         start=(k == 0), stop=(k == KH - 1))
                for c in range(ntc):
                    sg = ep.tile([P, 512], f32, tag="sg")
                    nc.scalar.activation(out=sg[:], in_=pgs[c][:],
                                         func=mybir.ActivationFunctionType.Silu)
                    nc.vector.tensor_mul(out=hts[i][:, c * 512:(c + 1) * 512],
                                         in0=sg[:], in1=pus[c][:])

            # ---- Stage B: y[tb,:] = sum_i (h^T tile)^T @ D[i], scaled by cw.
            # h^T tiles are stationary; D rows stream. Output is y [T, H], so
            # cw is a per-partition scalar (Scalar-engine scale / DVE
            # tensor_scalar), and no host transpose is needed.
            for g in range(nt // 2):
                tbs = (2 * g, 2 * g + 1)
                pys = [[ps.tile([P, 512], f32, tag=f"q{ti * NHC + hc}",
                                name=f"py{g}_{ti}_{hc}")
                        for hc in range(NHC)] for ti in range(2)]
                for i in range(NI):
                    for ti, tb in enumerate(tbs):
                        lh = hts[i][:, tb * P:(tb + 1) * P]
                        for hc in range(NHC):
                            nc.tensor.matmul(
                                out=pys[ti][hc][:], lhsT=lh,
                                rhs=dts[i][:, hc * 512:(hc + 1) * 512],
                                start=(i == 0), stop=(i == NI - 1))
                for ti, tb in enumerate(tbs):
                    for hc in range(NHC):
                        ysb = ep.tile([P, 512], f32, tag=f"yd{ti}{hc}")
                        if hc == 0:
                            nc.scalar.activation(
                                out=ysb[:], in_=pys[ti][hc][:],
                                func=mybir.ActivationFunctionType.Copy,
                                scale=cwt[:, tb:tb + 1])
                        else:
                            nc.vector.tensor_scalar_mul(
                                ysb[:], pys[ti][hc][:], cwt[:, tb:tb + 1])
                        eng = nc.gpsimd if ti else nc.sync
                        eng.dma_start(
                            out=y[tb * P:(tb + 1) * P,
                                  hc * 512:(hc + 1) * 512],
                            in_=ysb[:])

    _dedup_ldweights(nc)
    nc.finalize()
    return nc


def _dedup_ldweights(nc):
    """Drop an InstLdweights whose weights AP matches the immediately
    preceding load on the PE queue (matmuls between don't clobber the
    array). Saves the ~46ns/matmul the redundant reload steals from the
    PE issue pipeline. Only sync-free duplicates are removed."""
    import concourse.mybir as mybir

    def key(i):
        a = i.ins[0]
        return (a.memref, a.offset, tuple(map(tuple, a.ap)), str(a.dtype),
                str(i.perf_mode), str(i.is_transpose),
                str(getattr(i, "tile_position", None)))

    for blk in nc.m.functions[0].blocks:
        last = None
        keep = []
        for i in blk.instructions:
            if getattr(i, "engine", None) == mybir.EngineType.PE:
                if isinstance(i, mybir.InstLdweights):
                    k = key(i)
                    si = i.sync_info
                    clean = not (si and (si.on_wait or si.on_update))
                    if k == last and clean:
                        continue
                    last = k
                elif not isinstance(i, mybir.InstMatmult):
                    last = None
            keep.append(i)
        blk.instructions[:] = keep


def _route(expert_indices, expert_weights):
    idx = np.asarray(expert_indices).astype(np.int64)
    wts = np.asarray(expert_weights).astype(np.float32)
    n = idx.shape[0]
    cw_full = np.zeros((N_EXPERTS, n), np.float32)
    for k in range(idx.shape[1]):
        np.add.at(cw_full, (idx[:, k], np.arange(n)), wts[:, k])
    ids = [np.nonzero(cw_full[e])[0] for e in range(N_EXPERTS)]
    maxc = max(len(i) for i in ids)
    t_pad = max(1024, ((maxc + 1023) // 1024) * 1024)
    return cw_full, ids, t_pad


def _run(nc, in_maps, trace=False, trace_cores=None):
    from concourse.bass_utils import run_bass_kernel_spmd

    return run_bass_kernel_spmd(
        nc, in_maps, list(range(N_EXPERTS)), trace=trace,
        trace_cores=trace_cores,
    )


def prepare(tokens, expert_indices, expert_weights, gate_weight, up_weight,
            down_weight):
    """Host-side routing + layout. Returns (nc, in_maps, ids, t_pad)."""
    tokens = np.ascontiguousarray(np.asarray(tokens, dtype=np.float32))
    gate_weight = np.asarray(gate_weight, dtype=np.float32)
    up_weight = np.asarray(up_weight, dtype=np.float32)
    down_weight = np.asarray(down_weight, dtype=np.float32)

    cw_full, ids, t_pad = _route(expert_indices, expert_weights)
    nt = t_pad // P

    key = t_pad
    if key not in _CACHE:
        _CACHE[key] = _build(t_pad)
    nc = _CACHE[key]

    mmdt = np.dtype("bfloat16")
    in_maps = []
    for e in range(N_EXPERTS):
        ce = len(ids[e])
        xe = np.zeros((HIDDEN, t_pad), np.float32)
        xe[:, :ce] = tokens[ids[e]].T
        cwe = np.zeros((t_pad,), np.float32)
        cwe[:ce] = cw_full[e, ids[e]]
        in_maps.append({
            "xt": np.ascontiguousarray(xe.reshape(KH, P, t_pad)).astype(mmdt),
            "gw": np.ascontiguousarray(
                gate_weight[e].reshape(KH, P, NI, P).transpose(2, 1, 0, 3)
            ).reshape(NI, P, HIDDEN).astype(mmdt),
            "uw": np.ascontiguousarray(
                up_weight[e].reshape(KH, P, NI, P).transpose(2, 1, 0, 3)
            ).reshape(NI, P, HIDDEN).astype(mmdt),
            "dw": np.ascontiguousarray(down_weight[e].reshape(NI, P, HIDDEN)).astype(mmdt),
            "cw": np.ascontiguousarray(cwe.reshape(nt, P).T),
        })
    return nc, in_maps, ids, t_pad


def combine(results, ids):
    out = np.zeros((N_TOKENS, HIDDEN), np.float32)
    for e in range(N_EXPERTS):
        ce = len(ids[e])
        out[ids[e]] += results[e]["y"][:ce]
    return out


def kernel(tokens, expert_indices, expert_weights, gate_weight, up_weight,
           down_weight):
    nc, in_maps, ids, _ = prepare(tokens, expert_indices, expert_weights,
                                  gate_weight, up_weight, down_weight)
    res = _run(nc, in_maps, trace=False)
    return combine(res.results, ids)


# revision 7
# speedup vs baseline: 1.2679x; 1.0594x over previous
"""Expert-parallel MoE FFN kernel for Trainium2 (8 NeuronCores, one expert per core).

Host side: routes tokens to experts (dedup per expert, summing duplicate top-k
weights), pads each expert's token list to a common T_PAD, and pre-tiles the
weight matrices into DMA-friendly contiguous blocks.

Device side (per core, expert e):
  h^T = silu(G_e^T X^T) * (U_e^T X^T)     [I, T]   (stage A)
  y   = (h^T)^T @ D_e  * cw               [T, H]   (stage B; h^T tiles are the
                                           stationary operand so the cw combine
                                           becomes a per-partition scale)
All matmuls are bf16 with fp32 PSUM accumulation.

Perf structure:
 - DMA issue order puts the first tiles the PE needs (x^T k=0, gate/up i=0)
   at the head of the queue, prefetches the stage-B down-weights before the
   compute-gated gate/up stream, and defers the cw vector to last.
 - One shared PSUM pool (tags q0..q3, bufs=2) spans both stages, so stage B's
   accumulators take over stage A's bank ring without a pool barrier.
 - A post-schedule pass drops LDWEIGHTS reloads whose stationary tile is
   already in the PE array.
 - Stage B drains alternate Scalar/Vector engines and the output DMAs
   alternate Sync/GpSimd queues to shorten the kernel tail.
"""
import sys

if "/opt/trn_rl_repo" not in sys.path:
    sys.path.insert(0, "/opt/trn_rl_repo")

import numpy as np

N_TOKENS, TOP_K, N_EXPERTS, HIDDEN, INTER = 4096, 2, 8, 1024, 2048
P = 128
NI = INTER // P          # 16 I-tiles
KH = HIDDEN // P         # 8 H(contraction)-tiles
NHC = HIDDEN // 512      # 2 output-column chunks

_CACHE = {}


def _build(t_pad):
    import concourse.bacc as bacc
    import concourse.mybir as mybir
    import concourse.tile as tile

    f32 = mybir.dt.float32
    bf16 = mybir.dt.bfloat16

    nt = t_pad // P          # t-blocks of 128
    ntc = t_pad // 512       # t-chunks of 512
    assert ntc == 2 and nt % 2 == 0, "V2 layout assumes t_pad multiple of 1024"

    nc = bacc.Bacc()
    xt = nc.declare_dram_parameter("xt", [KH, P, t_pad], bf16, isOutput=False)
    gw = nc.declare_dram_parameter("gw", [NI, P, HIDDEN], bf16, isOutput=False)
    uw = nc.declare_dram_parameter("uw", [NI, P, HIDDEN], bf16, isOutput=False)
    dw = nc.declare_dram_parameter("dw", [NI, P, HIDDEN], bf16, isOutput=False)
    cw = nc.declare_dram_parameter("cw", [P, nt], f32, isOutput=False)
    y = nc.declare_dram_parameter("y", [t_pad, HIDDEN], f32, isOutput=True)

    with tile.TileContext(nc) as tc:
        with (
            tc.tile_pool(name="xp", bufs=1) as xp,
            tc.tile_pool(name="hp", bufs=1) as hp,
            tc.tile_pool(name="wp", bufs=2) as wp,
            tc.tile_pool(name="dp", bufs=1) as dp,
            tc.tile_pool(name="cp", bufs=1) as cp,
            tc.tile_pool(name="ep", bufs=3) as ep,
            tc.tile_pool(name="ps", bufs=2, space="PSUM") as ps,
        ):
            # ---- DMA issue order (single Sync queue ~ HBM-saturating):
            # stage-A critical path first, stage-B weights prefetched before
            # the compute-gated gate/up stream, cw (stage-B drain only) last.
            xts = [xp.tile([P, t_pad], bf16, tag=f"x{k}", name=f"xt{k}")
                   for k in range(KH)]
            nc.sync.dma_start(out=xts[0][:], in_=xt[0])
            wpre = []
            for i in range(2):
                gt = wp.tile([P, HIDDEN], bf16, tag="g", name=f"gt{i}")
                ut = wp.tile([P, HIDDEN], bf16, tag="u", name=f"ut{i}")
                nc.sync.dma_start(out=gt[:], in_=gw[i])
                nc.sync.dma_start(out=ut[:], in_=uw[i])
                wpre.append((gt, ut))
                if i == 0:
                    for k in range(1, KH):
                        nc.sync.dma_start(out=xts[k][:], in_=xt[k])
            dts = []
            for i in range(NI):
                dt_ = dp.tile([P, HIDDEN], bf16, tag=f"d{i}", name=f"dt{i}")
                nc.sync.dma_start(out=dt_[:], in_=dw[i])
                dts.append(dt_)
            cwt = cp.tile([P, nt], f32, name="cwt")
            nc.sync.dma_start(out=cwt[:], in_=cw[:])

            hts = [hp.tile([P, t_pad], bf16, tag=f"h{i}", name=f"ht{i}")
                   for i in range(NI)]

            # ---- Stage A: h^T[i] = silu(G^T X^T) * (U^T X^T), tiled over I ----
            for i in range(NI):
                if i < 2:
                    gt, ut = wpre[i]
                else:
                    gt = wp.tile([P, HIDDEN], bf16, tag="g", name=f"gt{i}")
                    ut = wp.tile([P, HIDDEN], bf16, tag="u", name=f"ut{i}")
                    nc.sync.dma_start(out=gt[:], in_=gw[i])
                    nc.sync.dma_start(out=ut[:], in_=uw[i])
                pgs = [ps.tile([P, 512], f32, tag=f"q{c}", name=f"pg{i}_{c}")
                       for c in range(ntc)]
                pus = [ps.tile([P, 512], f32, tag=f"q{ntc + c}", name=f"pu{i}_{c}")
                       for c in range(ntc)]
                for k in range(KH):
                    lg = gt[:, k * P:(k + 1) * P]
                    lu = ut[:, k * P:(k + 1) * P]
                    for c in range(ntc):
                        nc.tensor.matmul(out=pgs[c][:], lhsT=lg,
                                         rhs=xts[k][:, c * 512:(c + 1) * 512],
                                         start=(k == 0), stop=(k == KH - 1))
                    for c in range(ntc):
                        nc.tensor.matmul(out=pus[c][:], lhsT=lu,
                                         rhs=xts[k][:, c * 512:(c + 1) * 512],
                                         start=(k == 0), stop=(k == KH - 1))
                for c in range(ntc):
                    sg = ep.tile([P, 512], f32, tag="sg", name="sg")
                    nc.scalar.activation(out=sg[:], in_=pgs[c][:],
                                         func=mybir.ActivationFunctionType.Silu)
                    nc.vector.tensor_mul(out=hts[i][:, c * 512:(c + 1) * 512],
                                         in0=sg[:], in1=pus[c][:])

            # ---- Stage B: y[tb,:] = sum_i (h^T tile)^T @ D[i], scaled by cw.
            # h^T tiles are stationary; D rows stream. Output is y [T, H], so
            # cw is a per-partition scalar (Scalar-engine scale / DVE
            # tensor_scalar), and no host transpose is needed.
            for g in range(nt // 2):
                tbs = (2 * g, 2 * g + 1)
                pys = [[ps.tile([P, 512], f32, tag=f"q{ti * NHC + hc}",
                                name=f"py{g}_{ti}_{hc}")
                        for hc in range(NHC)] for ti in range(2)]
                for i in range(NI):
                    for ti, tb in enumerate(tbs):
                        lh = hts[i][:, tb * P:(tb + 1) * P]
                        for hc in range(NHC):
                            nc.tensor.matmul(
                                out=pys[ti][hc][:], lhsT=lh,
                                rhs=dts[i][:, hc * 512:(hc + 1) * 512],
                                start=(i == 0), stop=(i == NI - 1))
                for ti, tb in enumerate(tbs):
                    for hc in range(NHC):
                        ysb = ep.tile([P, 512], f32, tag=f"yd{ti}{hc}", name=f"ysb{ti}{hc}")
                        if hc == 0:
                            nc.scalar.activation(
                                out=ysb[:], in_=pys[ti][hc][:],
                                func=mybir.ActivationFunctionType.Copy,
                                scale=cwt[:, tb:tb + 1])
                        else:
                            nc.vector.tensor_scalar_mul(
                                ysb[:], pys[ti][hc][:], cwt[:, tb:tb + 1])
                        eng = nc.gpsimd if ti else nc.sync
                        eng.dma_start(
                            out=y[tb * P:(tb + 1) * P,
                                  hc * 512:(hc + 1) * 512],
                            in_=ysb[:])

    _dedup_ldweights(nc)
    nc.finalize()
    return nc


def _dedup_ldweights(nc):
    """Drop an InstLdweights whose weights AP matches the immediately
    preceding load on the PE queue (matmuls between don't clobber the
    array). Saves the ~46ns/matmul the redundant reload steals from the
    PE issue pipeline. Only sync-free duplicates are removed."""
    import concourse.mybir as mybir

    def key(i):
        a = i.ins[0]
        return (a.memref, a.offset, tuple(map(tuple, a.ap)), str(a.dtype),
                str(i.perf_mode), str(i.is_transpose),
                str(getattr(i, "tile_position", None)))

    for blk in nc.m.functions[0].blocks:
        last = None
        keep = []
        for i in blk.instructions:
            if getattr(i, "engine", None) == mybir.EngineType.PE:
                if isinstance(i, mybir.InstLdweights):
                    k = key(i)
                    si = i.sync_info
                    clean = not (si and (si.on_wait or si.on_update))
                    if k == last and clean:
                        continue
                    last = k
                elif not isinstance(i, mybir.InstMatmult):
                    last = None
            keep.append(i)
        blk.instructions[:] = keep


def _route(expert_indices, expert_weights):
    idx = np.asarray(expert_indices).astype(np.int64)
    wts = np.asarray(expert_weights).astype(np.float32)
    n = idx.shape[0]
    cw_full = np.zeros((N_EXPERTS, n), np.float32)
    for k in range(idx.shape[1]):
        np.add.at(cw_full, (idx[:, k], np.arange(n)), wts[:, k])
    ids = [np.nonzero(cw_full[e])[0] for e in range(N_EXPERTS)]
    maxc = max(len(i) for i in ids)
    t_pad = max(1024, ((maxc + 1023) // 1024) * 1024)
    return cw_full, ids, t_pad


def _run(nc, in_maps, trace=False, trace_cores=None):
    from concourse.bass_utils import run_bass_kernel_spmd

    return run_bass_kernel_spmd(
        nc, in_maps, list(range(N_EXPERTS)), trace=trace,
        trace_cores=trace_cores,
    )


def prepare(tokens, expert_indices, expert_weights, gate_weight, up_weight,
            down_weight):
    """Host-side routing + layout. Returns (nc, in_maps, ids, t_pad)."""
    tokens = np.ascontiguousarray(np.asarray(tokens, dtype=np.float32))
    gate_weight = np.asarray(gate_weight, dtype=np.float32)
    up_weight = np.asarray(up_weight, dtype=np.float32)
    down_weight = np.asarray(down_weight, dtype=np.float32)

    cw_full, ids, t_pad = _route(expert_indices, expert_weights)
    nt = t_pad // P

    key = t_pad
    if key not in _CACHE:
        _CACHE[key] = _build(t_pad)
    nc = _CACHE[key]

    mmdt = np.dtype("bfloat16")
    in_maps = []
    for e in range(N_EXPERTS):
        ce = len(ids[e])
        xe = np.zeros((HIDDEN, t_pad), np.float32)
        xe[:, :ce] = tokens[ids[e]].T
        cwe = np.zeros((t_pad,), np.float32)
        cwe[:ce] = cw_full[e, ids[e]]
        in_maps.append({
            "xt": np.ascontiguousarray(xe.reshape(KH, P, t_pad)).astype(mmdt),
            "gw": np.ascontiguousarray(
                gate_weight[e].reshape(KH, P, NI, P).transpose(2, 1, 0, 3)
            ).reshape(NI, P, HIDDEN).astype(mmdt),
            "uw": np.ascontiguousarray(
                up_weight[e].reshape(KH, P, NI, P).transpose(2, 1, 0, 3)
            ).reshape(NI, P, HIDDEN).astype(mmdt),
            "dw": np.ascontiguousarray(down_weight[e].reshape(NI, P, HIDDEN)).astype(mmdt),
            "cw": np.ascontiguousarray(cwe.reshape(nt, P).T),
        })
    return nc, in_maps, ids, t_pad


def combine(results, ids):
    out = np.zeros((N_TOKENS, HIDDEN), np.float32)
    for e in range(N_EXPERTS):
        ce = len(ids[e])
        out[ids[e]] += results[e]["y"][:ce]
    return out


def kernel(tokens, expert_indices, expert_weights, gate_weight, up_weight,
           down_weight):
    nc, in_maps, ids, _ = prepare(tokens, expert_indices, expert_weights,
                                  gate_weight, up_weight, down_weight)
    res = _run(nc, in_maps, trace=False)
    return combine(res.results, ids)


# revision 20
# speedup vs baseline: 1.3242x; 1.0444x over previous
"""Expert-parallel MoE FFN kernel for Trainium2 (8 NeuronCores, one expert per core).

Host side: routes tokens to experts (dedup per expert, summing duplicate top-k
weights), pads each expert's token list to a common T_PAD, and pre-tiles the
weight matrices into DMA-friendly contiguous blocks.

Device side (per core, expert e):
  h^T = silu(G_e^T X^T) * (U_e^T X^T)     [I, T]   (stage A)
  y   = (h^T)^T @ D_e  * cw               [T, H]   (stage B; h^T tiles are the
                                           stationary operand so the cw combine
                                           becomes a per-partition scale)
All matmuls are bf16 with fp32 PSUM accumulation.

Perf structure:
 - DMA issue order puts the first tiles the PE needs (x^T k=0, gate/up i=0)
   at the head of the queue, prefetches the stage-B down-weights before the
   compute-gated gate/up stream, and defers the cw vector to last.
 - One shared PSUM pool (tags q0..q3, bufs=2) spans both stages, so stage B's
   accumulators take over stage A's bank ring without a pool barrier.
 - A post-schedule pass drops LDWEIGHTS reloads whose stationary tile is
   already in the PE array.
 - Stage B drains alternate Scalar/Vector engines and the output DMAs
   alternate Sync/GpSimd queues to shorten the kernel tail.
"""
import sys

if "/opt/trn_rl_repo" not in sys.path:
    sys.path.insert(0, "/opt/trn_rl_repo")

import numpy as np

N_TOKENS, TOP_K, N_EXPERTS, HIDDEN, INTER = 4096, 2, 8, 1024, 2048
P = 128
NI = INTER // P          # 16 I-tiles
KH = HIDDEN // P         # 8 H(contraction)-tiles
NHC = HIDDEN // 512      # 2 output-column chunks

_CACHE = {}


def _build(t_pad):
    import concourse.bacc as bacc
    import concourse.mybir as mybir
    import concourse.tile as tile

    f32 = mybir.dt.float32
    bf16 = mybir.dt.bfloat16

    nt = t_pad // P          # t-blocks of 128
    ntc = t_pad // 512       # t-chunks of 512
    assert ntc == 2 and nt % 2 == 0, "V2 layout assumes t_pad multiple of 1024"

    nc = bacc.Bacc()
    xt = nc.declare_dram_parameter("xt", [KH, P, t_pad], bf16, isOutput=False)
    gw = nc.declare_dram_parameter("gw", [NI, P, HIDDEN], bf16, isOutput=False)
    uw = nc.declare_dram_parameter("uw", [NI, P, HIDDEN], bf16, isOutput=False)
    dw = nc.declare_dram_parameter("dw", [NI, P, HIDDEN], bf16, isOutput=False)
    cw = nc.declare_dram_parameter("cw", [P, nt], f32, isOutput=False)
    y = nc.declare_dram_parameter("y", [t_pad, HIDDEN], bf16, isOutput=True)

    with tile.TileContext(nc) as tc:
        with (
            tc.tile_pool(name="xp", bufs=1) as xp,
            tc.tile_pool(name="hp", bufs=1) as hp,
            tc.tile_pool(name="wp", bufs=4) as wp,
            tc.tile_pool(name="dp", bufs=1) as dp,
            tc.tile_pool(name="cp", bufs=1) as cp,
            tc.tile_pool(name="ep", bufs=3) as ep,
            tc.tile_pool(name="sp", bufs=1) as sp,
            tc.tile_pool(name="ps", bufs=2, space="PSUM") as ps,
        ):
            # ---- HAM warm-up: dummy matmuls on a memset scratch tile run
            # during the input-DMA head so the PE clock is at 8/8 before the
            # first real matmul (saves the ~3.4us half-rate ramp).
            scratch = sp.tile([P, P], bf16, name="scratch")
            nc.gpsimd.memset(scratch[:], 0)
            wps = ps.tile([P, 512], f32, tag="q0", name="warm")
            for _ in range(40):
                nc.tensor.matmul(out=wps[:, 0:P], lhsT=scratch[:],
                                 rhs=scratch[:], start=True, stop=True)

            # ---- DMA issue order (single Sync queue ~ HBM-saturating):
            # stage-A critical path first, stage-B weights prefetched before
            # the compute-gated gate/up stream, cw (stage-B drain only) last.
            xts = [xp.tile([P, t_pad], bf16, tag=f"x{k}", name=f"xt{k}")
                   for k in range(KH)]
            nc.sync.dma_start(out=xts[0][:], in_=xt[0])
            wpre = []
            for i in range(4):
                gt = wp.tile([P, HIDDEN], bf16, tag="g", name=f"gt{i}")
                ut = wp.tile([P, HIDDEN], bf16, tag="u", name=f"ut{i}")
                if i == 0:
                    # first gate/up tiles ride their own engine queues so the
                    # three head-critical transfers generate descriptors in
                    # parallel instead of serializing on Sync
                    nc.scalar.dma_start(out=gt[:], in_=gw[i])
                    nc.gpsimd.dma_start(out=ut[:], in_=uw[i])
                    for k in range(1, KH):
                        nc.sync.dma_start(out=xts[k][:], in_=xt[k])
                else:
                    nc.sync.dma_start(out=gt[:], in_=gw[i])
                    nc.sync.dma_start(out=ut[:], in_=uw[i])
                wpre.append((gt, ut))
            dts = [dp.tile([P, HIDDEN], bf16, tag=f"d{i}", name=f"dt{i}")
                   for i in range(NI)]
            cwt = cp.tile([P, nt], f32, name="cwt")
            nc.scalar.dma_start(out=cwt[:], in_=cw[:])

            hts = [hp.tile([P, t_pad], bf16, tag=f"h{i}", name=f"ht{i}")
                   for i in range(NI)]

            # ---- Stage A: h^T[i] = silu(G^T X^T) * (U^T X^T), tiled over I ----
            for i in range(NI):
                if i < 4:
                    gt, ut = wpre[i]
                else:
                    gt = wp.tile([P, HIDDEN], bf16, tag="g", name=f"gt{i}")
                    ut = wp.tile([P, HIDDEN], bf16, tag="u", name=f"ut{i}")
                    nc.sync.dma_start(out=gt[:], in_=gw[i])
                    nc.sync.dma_start(out=ut[:], in_=uw[i])
                nc.sync.dma_start(out=dts[i][:], in_=dw[i])
                pgs = [ps.tile([P, 512], f32, tag=f"q{c}", name=f"pg{i}_{c}")
                       for c in range(ntc)]
                pus = [ps.tile([P, 512], f32, tag=f"q{ntc + c}", name=f"pu{i}_{c}")
                       for c in range(ntc)]
                for k in range(KH):
                    lg = gt[:, k * P:(k + 1) * P]
                    lu = ut[:, k * P:(k + 1) * P]
                    for c in range(ntc):
                        nc.tensor.matmul(out=pgs[c][:], lhsT=lg,
                                         rhs=xts[k][:, c * 512:(c + 1) * 512],
                                         start=(k == 0), stop=(k == KH - 1))
                    for c in range(ntc):
                        nc.tensor.matmul(out=pus[c][:], lhsT=lu,
                                         rhs=xts[k][:, c * 512:(c + 1) * 512],
                                         start=(k == 0), stop=(k == KH - 1))
                for c in range(ntc):
                    sg = ep.tile([P, 512], f32, tag="sg", name="sg")
                    nc.scalar.activation(out=sg[:], in_=pgs[c][:],
                                         func=mybir.ActivationFunctionType.Silu)
                    nc.vector.tensor_mul(out=hts[i][:, c * 512:(c + 1) * 512],
                                         in0=sg[:], in1=pus[c][:])

            # ---- Stage B: y[tb,:] = sum_i (h^T tile)^T @ D[i], scaled by cw.
            # h^T tiles are stationary; D rows stream. Output is y [T, H], so
            # cw is a per-partition scalar (Scalar-engine scale / DVE
            # tensor_scalar), and no host transpose is needed.
            for g in range(nt // 2):
                tbs = (2 * g, 2 * g + 1)
                pys = [[ps.tile([P, 512], f32, tag=f"q{ti * NHC + hc}",
                                name=f"py{g}_{ti}_{hc}")
                        for hc in range(NHC)] for ti in range(2)]
                for i in range(NI):
                    for ti, tb in enumerate(tbs):
                        lh = hts[i][:, tb * P:(tb + 1) * P]
                        for hc in range(NHC):
                            nc.tensor.matmul(
                                out=pys[ti][hc][:], lhsT=lh,
                                rhs=dts[i][:, hc * 512:(hc + 1) * 512],
                                start=(i == 0), stop=(i == NI - 1))
                for ti, tb in enumerate(tbs):
                    ybt = ep.tile([P, HIDDEN], bf16, tag=f"yd{ti}",
                                  name=f"ybt{ti}")
                    nc.scalar.activation(
                        out=ybt[:, 0:512], in_=pys[ti][0][:],
                        func=mybir.ActivationFunctionType.Copy,
                        scale=cwt[:, tb:tb + 1])
                    nc.vector.tensor_scalar_mul(
                        ybt[:, 512:HIDDEN], pys[ti][1][:], cwt[:, tb:tb + 1])
                    eng = nc.sync if ti == 0 else nc.scalar
                    eng.dma_start(out=y[tb * P:(tb + 1) * P, :], in_=ybt[:])

    _dedup_ldweights(nc)
    _hoist_pe_waits(nc)
    nc.finalize()
    return nc


def _chain_head_dmas(nc):
    """Gate the bulk input stream (x1 onward) behind the completion of the
    three head-critical transfers (x0 implicitly by queue order, g0/u0 by an
    added wait). HW DGE queues are shared round-robin across engines, so
    without this the bulk transfers steal ~2/3 of the HBM bandwidth while
    the PE is still waiting for its first tiles."""
    import concourse.mybir as mybir

    for blk in nc.m.functions[0].blocks:
        g0 = u0 = x1 = None
        for i in blk.instructions:
            if not isinstance(i, mybir.InstDMACopy):
                continue
            m = getattr(i.outs[0], "memref", "") or ""
            if m.startswith("gt0"):
                g0 = i
            elif m.startswith("ut0"):
                u0 = i
            elif m.startswith("xt1_") or m == "xt1":
                x1 = i
        if not (g0 and u0 and x1):
            continue
        waits = []
        for d in (g0, u0):
            si = d.sync_info
            if not (si and si.on_update):
                continue
            u = si.on_update[0]
            waits.append(mybir.SyncWait(
                sync_type="semaphore", id=u.id, ant_name=u.ant_name,
                wait_mode="sem-ge-imm", wait_value=16, wait_reg=None))
        if waits:
            xsi = x1.sync_info or mybir.SyncInfo(on_wait=[], on_update=[])
            xsi.on_wait = list(xsi.on_wait) + waits
            x1.sync_info = xsi


def _hoist_pe_waits(nc, dist=8, skip=42):
    """Move semaphore waits off PE matmul/ldweights instructions into a
    standalone EVENT_SEMAPHORE `dist` engine-instructions earlier. A bare
    LDWEIGHTS can be pulled ahead of in-flight matmuls by the PE's reorder
    window; a wait-carrying one cannot (measured: 432ns vs 213ns pacing at
    every weight-ring boundary). All hoisted waits are prefetch-satisfied
    long before the insertion point. The first `skip` engine instructions
    (HAM warm-up dummies + first loads) keep their waits in place."""
    import concourse.mybir as mybir
    from collections import defaultdict

    for blk in nc.m.functions[0].blocks:
        pe_pos = [bi for bi, i in enumerate(blk.instructions)
                  if getattr(i, "engine", None) == mybir.EngineType.PE
                  and isinstance(i, (mybir.InstMatmult, mybir.InstLdweights))]
        if len(pe_pos) < skip:
            continue
        inserts = []  # (block_index, evsem)
        for j, bi in enumerate(pe_pos):
            if j < skip:
                continue
            inst = blk.instructions[bi]
            si = inst.sync_info
            if not (si and si.on_wait):
                continue
            target = pe_pos[max(skip, j - dist)]
            if target >= bi:
                continue
            for w in si.on_wait:
                # DMA-completion waits (input prefetches, always long
                # satisfied) go `dist` engine-instructions early; engine-sem
                # waits (PSUM WAR etc.) split to an EVSEM immediately before
                # the instruction -- same queue position, but the LDW/MM
                # itself becomes bare and eligible for pull-ahead.
                t = target if "DMA" in (w.ant_name or "") else bi
                ev = mybir.InstEventSemaphore(
                    name=nc.get_next_instruction_name(), ins=[], outs=[])
                ev.engine = mybir.EngineType.PE
                ev.sync_info = mybir.SyncInfo(on_wait=[w], on_update=[])
                nc.register_instruction(ev)
                inserts.append((t, ev))
            si.on_wait = []
        if not inserts:
            continue
        by_idx = defaultdict(list)
        for t, ev in inserts:
            by_idx[t].append(ev)
        out = []
        for bi, inst in enumerate(blk.instructions):
            if bi in by_idx:
                out.extend(by_idx[bi])
            out.append(inst)
        blk.instructions[:] = out


def _dedup_ldweights(nc):
    """Drop an InstLdweights whose weights AP matches the immediately
    preceding load on the PE queue (matmuls between don't clobber the
    array). Saves the ~46ns/matmul the redundant reload steals from the
    PE issue pipeline. Only sync-free duplicates are removed."""
    import concourse.mybir as mybir

    def key(i):
        a = i.ins[0]
        return (a.memref, a.offset, tuple(map(tuple, a.ap)), str(a.dtype),
                str(i.perf_mode), str(i.is_transpose),
                str(getattr(i, "tile_position", None)))

    for blk in nc.m.functions[0].blocks:
        last = None
        keep = []
        for i in blk.instructions:
            if getattr(i, "engine", None) == mybir.EngineType.PE:
                if isinstance(i, mybir.InstLdweights):
                    k = key(i)
                    si = i.sync_info
                    clean = not (si and (si.on_wait or si.on_update))
                    if k == last and clean:
                        continue
                    last = k
                elif not isinstance(i, mybir.InstMatmult):
                    last = None
            keep.append(i)
        blk.instructions[:] = keep


def _route(expert_indices, expert_weights):
    idx = np.asarray(expert_indices).astype(np.int64)
    wts = np.asarray(expert_weights).astype(np.float32)
    n = idx.shape[0]
    cw_full = np.zeros((N_EXPERTS, n), np.float32)
    for k in range(idx.shape[1]):
        np.add.at(cw_full, (idx[:, k], np.arange(n)), wts[:, k])
    ids = [np.nonzero(cw_full[e])[0] for e in range(N_EXPERTS)]
    maxc = max(len(i) for i in ids)
    t_pad = max(1024, ((maxc + 1023) // 1024) * 1024)
    return cw_full, ids, t_pad


def _run(nc, in_maps, trace=False, trace_cores=None):
    from concourse.bass_utils import run_bass_kernel_spmd

    return run_bass_kernel_spmd(
        nc, in_maps, list(range(N_EXPERTS)), trace=trace,
        trace_cores=trace_cores,
    )


def prepare(tokens, expert_indices, expert_weights, gate_weight, up_weight,
            down_weight):
    """Host-side routing + layout. Returns (nc, in_maps, ids, t_pad)."""
    tokens = np.ascontiguousarray(np.asarray(tokens, dtype=np.float32))
    gate_weight = np.asarray(gate_weight, dtype=np.float32)
    up_weight = np.asarray(up_weight, dtype=np.float32)
    down_weight = np.asarray(down_weight, dtype=np.float32)

    cw_full, ids, t_pad = _route(expert_indices, expert_weights)
    nt = t_pad // P

    key = t_pad
    if key not in _CACHE:
        _CACHE[key] = _build(t_pad)
    nc = _CACHE[key]

    mmdt = np.dtype("bfloat16")
    in_maps = []
    for e in range(N_EXPERTS):
        ce = len(ids[e])
        xe = np.zeros((HIDDEN, t_pad), np.float32)
        xe[:, :ce] = tokens[ids[e]].T
        cwe = np.zeros((t_pad,), np.float32)
        cwe[:ce] = cw_full[e, ids[e]]
        in_maps.append({
            "xt": np.ascontiguousarray(xe.reshape(KH, P, t_pad)).astype(mmdt),
            "gw": np.ascontiguousarray(
                gate_weight[e].reshape(KH, P, NI, P).transpose(2, 1, 0, 3)
            ).reshape(NI, P, HIDDEN).astype(mmdt),
            "uw": np.ascontiguousarray(
                up_weight[e].reshape(KH, P, NI, P).transpose(2, 1, 0, 3)
            ).reshape(NI, P, HIDDEN).astype(mmdt),
            "dw": np.ascontiguousarray(down_weight[e].reshape(NI, P, HIDDEN)).astype(mmdt),
            "cw": np.ascontiguousarray(cwe.reshape(nt, P).T),
        })
    return nc, in_maps, ids, t_pad


def combine(results, ids):
    out = np.zeros((N_TOKENS, HIDDEN), np.float32)
    for e in range(N_EXPERTS):
        ce = len(ids[e])
        out[ids[e]] += results[e]["y"][:ce].astype(np.float32)
    return out


def kernel(tokens, expert_indices, expert_weights, gate_weight, up_weight,
           down_weight):
    nc, in_maps, ids, _ = prepare(tokens, expert_indices, expert_weights,
                                  gate_weight, up_weight, down_weight)
    res = _run(nc, in_maps, trace=False)
    return combine(res.results, ids)


# revision 21
# speedup vs baseline: 1.3248x; 1.0005x over previous
"""Expert-parallel MoE FFN kernel for Trainium2 (8 NeuronCores, one expert per core).

Host side: routes tokens to experts (dedup per expert, summing duplicate top-k
weights), pads each expert's token list to a common T_PAD, and pre-tiles the
weight matrices into DMA-friendly contiguous blocks.

Device side (per core, expert e):
  h^T = silu(G_e^T X^T) * (U_e^T X^T)     [I, T]   (stage A)
  y   = (h^T)^T @ D_e  * cw               [T, H]   (stage B; h^T tiles are the
                                           stationary operand so the cw combine
                                           becomes a per-partition scale)
All matmuls are bf16 with fp32 PSUM accumulation.

Perf structure:
 - DMA issue order puts the first tiles the PE needs (x^T k=0, gate/up i=0)
   at the head of the queue, prefetches the stage-B down-weights before the
   compute-gated gate/up stream, and defers the cw vector to last.
 - One shared PSUM pool (tags q0..q3, bufs=2) spans both stages, so stage B's
   accumulators take over stage A's bank ring without a pool barrier.
 - A post-schedule pass drops LDWEIGHTS reloads whose stationary tile is
   already in the PE array.
 - Stage B drains alternate Scalar/Vector engines and the output DMAs
   alternate Sync/GpSimd queues to shorten the kernel tail.
"""
import sys

if "/opt/trn_rl_repo" not in sys.path:
    sys.path.insert(0, "/opt/trn_rl_repo")

import numpy as np

N_TOKENS, TOP_K, N_EXPERTS, HIDDEN, INTER = 4096, 2, 8, 1024, 2048
P = 128
NI = INTER // P          # 16 I-tiles
KH = HIDDEN // P         # 8 H(contraction)-tiles
NHC = HIDDEN // 512      # 2 output-column chunks

_CACHE = {}


def _build(t_pad):
    import concourse.bacc as bacc
    import concourse.mybir as mybir
    import concourse.tile as tile

    f32 = mybir.dt.float32
    bf16 = mybir.dt.bfloat16

    nt = t_pad // P          # t-blocks of 128
    ntc = t_pad // 512       # t-chunks of 512
    assert ntc == 2 and nt % 2 == 0, "V2 layout assumes t_pad multiple of 1024"

    nc = bacc.Bacc()
    xt = nc.declare_dram_parameter("xt", [KH, P, t_pad], bf16, isOutput=False)
    gw = nc.declare_dram_parameter("gw", [NI, P, HIDDEN], bf16, isOutput=False)
    uw = nc.declare_dram_parameter("uw", [NI, P, HIDDEN], bf16, isOutput=False)
    dw = nc.declare_dram_parameter("dw", [NI, P, HIDDEN], bf16, isOutput=False)
    cw = nc.declare_dram_parameter("cw", [P, nt], f32, isOutput=False)
    y = nc.declare_dram_parameter("y", [t_pad, HIDDEN], bf16, isOutput=True)

    with tile.TileContext(nc) as tc:
        with (
            tc.tile_pool(name="xp", bufs=1) as xp,
            tc.tile_pool(name="hp", bufs=1) as hp,
            tc.tile_pool(name="wp", bufs=4) as wp,
            tc.tile_pool(name="dp", bufs=1) as dp,
            tc.tile_pool(name="cp", bufs=1) as cp,
            tc.tile_pool(name="ep", bufs=3) as ep,
            tc.tile_pool(name="sp", bufs=1) as sp,
            tc.tile_pool(name="ps", bufs=2, space="PSUM") as ps,
        ):
            # ---- HAM warm-up: dummy matmuls on a memset scratch tile run
            # during the input-DMA head so the PE clock is at 8/8 before the
            # first real matmul (saves the ~3.4us half-rate ramp).
            scratch = sp.tile([P, 512], bf16, name="scratch")
            nc.gpsimd.memset(scratch[:], 0)
            wps = ps.tile([P, 512], f32, tag="q0", name="warm")
            for _ in range(8):
                nc.tensor.matmul(out=wps[:], lhsT=scratch[:, 0:P],
                                 rhs=scratch[:], start=True, stop=True)

            # ---- DMA issue order (single Sync queue ~ HBM-saturating):
            # stage-A critical path first, stage-B weights prefetched before
            # the compute-gated gate/up stream, cw (stage-B drain only) last.
            xts = [xp.tile([P, t_pad], bf16, tag=f"x{k}", name=f"xt{k}")
                   for k in range(KH)]
            nc.sync.dma_start(out=xts[0][:], in_=xt[0])
            wpre = []
            for i in range(4):
                gt = wp.tile([P, HIDDEN], bf16, tag="g", name=f"gt{i}")
                ut = wp.tile([P, HIDDEN], bf16, tag="u", name=f"ut{i}")
                if i == 0:
                    # first gate/up tiles ride their own engine queues so the
                    # three head-critical transfers generate descriptors in
                    # parallel instead of serializing on Sync
                    nc.scalar.dma_start(out=gt[:], in_=gw[i])
                    nc.gpsimd.dma_start(out=ut[:], in_=uw[i])
                    for k in range(1, KH):
                        nc.sync.dma_start(out=xts[k][:], in_=xt[k])
                else:
                    nc.sync.dma_start(out=gt[:], in_=gw[i])
                    nc.sync.dma_start(out=ut[:], in_=uw[i])
                wpre.append((gt, ut))
            dts = [dp.tile([P, HIDDEN], bf16, tag=f"d{i}", name=f"dt{i}")
                   for i in range(NI)]
            cwt = cp.tile([P, nt], f32, name="cwt")
            nc.scalar.dma_start(out=cwt[:], in_=cw[:])

            hts = [hp.tile([P, t_pad], bf16, tag=f"h{i}", name=f"ht{i}")
                   for i in range(NI)]

            # ---- Stage A: h^T[i] = silu(G^T X^T) * (U^T X^T), tiled over I ----
            for i in range(NI):
                if i < 4:
                    gt, ut = wpre[i]
                else:
                    gt = wp.tile([P, HIDDEN], bf16, tag="g", name=f"gt{i}")
                    ut = wp.tile([P, HIDDEN], bf16, tag="u", name=f"ut{i}")
                    nc.sync.dma_start(out=gt[:], in_=gw[i])
                    nc.sync.dma_start(out=ut[:], in_=uw[i])
                nc.sync.dma_start(out=dts[i][:], in_=dw[i])
                pgs = [ps.tile([P, 512], f32, tag=f"q{c}", name=f"pg{i}_{c}")
                       for c in range(ntc)]
                pus = [ps.tile([P, 512], f32, tag=f"q{ntc + c}", name=f"pu{i}_{c}")
                       for c in range(ntc)]
                for k in range(KH):
                    lg = gt[:, k * P:(k + 1) * P]
                    lu = ut[:, k * P:(k + 1) * P]
                    for c in range(ntc):
                        nc.tensor.matmul(out=pgs[c][:], lhsT=lg,
                                         rhs=xts[k][:, c * 512:(c + 1) * 512],
                                         start=(k == 0), stop=(k == KH - 1))
                    for c in range(ntc):
                        nc.tensor.matmul(out=pus[c][:], lhsT=lu,
                                         rhs=xts[k][:, c * 512:(c + 1) * 512],
                                         start=(k == 0), stop=(k == KH - 1))
                for c in range(ntc):
                    sg = ep.tile([P, 512], f32, tag="sg", name="sg")
                    nc.scalar.activation(out=sg[:], in_=pgs[c][:],
                                         func=mybir.ActivationFunctionType.Silu)
                    nc.vector.tensor_mul(out=hts[i][:, c * 512:(c + 1) * 512],
                                         in0=sg[:], in1=pus[c][:])

            # ---- Stage B: y[tb,:] = sum_i (h^T tile)^T @ D[i], scaled by cw.
            # h^T tiles are stationary; D rows stream. Output is y [T, H], so
            # cw is a per-partition scalar (Scalar-engine scale / DVE
            # tensor_scalar), and no host transpose is needed.
            for g in range(nt // 2):
                tbs = (2 * g, 2 * g + 1)
                pys = [[ps.tile([P, 512], f32, tag=f"q{ti * NHC + hc}",
                                name=f"py{g}_{ti}_{hc}")
                        for hc in range(NHC)] for ti in range(2)]
                for i in range(NI):
                    for ti, tb in enumerate(tbs):
                        lh = hts[i][:, tb * P:(tb + 1) * P]
                        for hc in range(NHC):
                            nc.tensor.matmul(
                                out=pys[ti][hc][:], lhsT=lh,
                                rhs=dts[i][:, hc * 512:(hc + 1) * 512],
                                start=(i == 0), stop=(i == NI - 1))
                for ti, tb in enumerate(tbs):
                    ybt = ep.tile([P, HIDDEN], bf16, tag=f"yd{ti}",
                                  name=f"ybt{ti}")
                    nc.scalar.activation(
                        out=ybt[:, 0:512], in_=pys[ti][0][:],
                        func=mybir.ActivationFunctionType.Copy,
                        scale=cwt[:, tb:tb + 1])
                    nc.vector.tensor_scalar_mul(
                        ybt[:, 512:HIDDEN], pys[ti][1][:], cwt[:, tb:tb + 1])
                    eng = nc.sync if ti == 0 else nc.scalar
                    eng.dma_start(out=y[tb * P:(tb + 1) * P, :], in_=ybt[:])

    _dedup_ldweights(nc)
    _hoist_pe_waits(nc)
    nc.finalize()
    return nc


def _chain_head_dmas(nc):
    """Gate the bulk input stream (x1 onward) behind the completion of the
    three head-critical transfers (x0 implicitly by queue order, g0/u0 by an
    added wait). HW DGE queues are shared round-robin across engines, so
    without this the bulk transfers steal ~2/3 of the HBM bandwidth while
    the PE is still waiting for its first tiles."""
    import concourse.mybir as mybir

    for blk in nc.m.functions[0].blocks:
        g0 = u0 = x1 = None
        for i in blk.instructions:
            if not isinstance(i, mybir.InstDMACopy):
                continue
            m = getattr(i.outs[0], "memref", "") or ""
            if m.startswith("gt0"):
                g0 = i
            elif m.startswith("ut0"):
                u0 = i
            elif m.startswith("xt1_") or m == "xt1":
                x1 = i
        if not (g0 and u0 and x1):
            continue
        waits = []
        for d in (g0, u0):
            si = d.sync_info
            if not (si and si.on_update):
                continue
            u = si.on_update[0]
            waits.append(mybir.SyncWait(
                sync_type="semaphore", id=u.id, ant_name=u.ant_name,
                wait_mode="sem-ge-imm", wait_value=16, wait_reg=None))
        if waits:
            xsi = x1.sync_info or mybir.SyncInfo(on_wait=[], on_update=[])
            xsi.on_wait = list(xsi.on_wait) + waits
            x1.sync_info = xsi


def _hoist_pe_waits(nc, dist=8, skip=42):
    """Move semaphore waits off PE matmul/ldweights instructions into a
    standalone EVENT_SEMAPHORE `dist` engine-instructions earlier. A bare
    LDWEIGHTS can be pulled ahead of in-flight matmuls by the PE's reorder
    window; a wait-carrying one cannot (measured: 432ns vs 213ns pacing at
    every weight-ring boundary). All hoisted waits are prefetch-satisfied
    long before the insertion point. The first `skip` engine instructions
    (HAM warm-up dummies + first loads) keep their waits in place."""
    import concourse.mybir as mybir
    from collections import defaultdict

    for blk in nc.m.functions[0].blocks:
        pe_pos = [bi for bi, i in enumerate(blk.instructions)
                  if getattr(i, "engine", None) == mybir.EngineType.PE
                  and isinstance(i, (mybir.InstMatmult, mybir.InstLdweights))]
        if len(pe_pos) < skip:
            continue
        inserts = []  # (block_index, evsem)
        for j, bi in enumerate(pe_pos):
            if j < skip:
                continue
            inst = blk.instructions[bi]
            si = inst.sync_info
            if not (si and si.on_wait):
                continue
            target = pe_pos[max(skip, j - dist)]
            if target >= bi:
                continue
            for w in si.on_wait:
                # DMA-completion waits (input prefetches, always long
                # satisfied) go `dist` engine-instructions early; engine-sem
                # waits (PSUM WAR etc.) split to an EVSEM immediately before
                # the instruction -- same queue position, but the LDW/MM
                # itself becomes bare and eligible for pull-ahead.
                t = target if "DMA" in (w.ant_name or "") else bi
                ev = mybir.InstEventSemaphore(
                    name=nc.get_next_instruction_name(), ins=[], outs=[])
                ev.engine = mybir.EngineType.PE
                ev.sync_info = mybir.SyncInfo(on_wait=[w], on_update=[])
                nc.register_instruction(ev)
                inserts.append((t, ev))
            si.on_wait = []
        if not inserts:
            continue
        by_idx = defaultdict(list)
        for t, ev in inserts:
            by_idx[t].append(ev)
        out = []
        for bi, inst in enumerate(blk.instructions):
            if bi in by_idx:
                out.extend(by_idx[bi])
            out.append(inst)
        blk.instructions[:] = out


def _dedup_ldweights(nc):
    """Drop an InstLdweights whose weights AP matches the immediately
    preceding load on the PE queue (matmuls between don't clobber the
    array). Saves the ~46ns/matmul the redundant reload steals from the
    PE issue pipeline. Only sync-free duplicates are removed."""
    import concourse.mybir as mybir

    def key(i):
        a = i.ins[0]
        return (a.memref, a.offset, tuple(map(tuple, a.ap)), str(a.dtype),
                str(i.perf_mode), str(i.is_transpose),
                str(getattr(i, "tile_position", None)))

    for blk in nc.m.functions[0].blocks:
        last = None
        keep = []
        for i in blk.instructions:
            if getattr(i, "engine", None) == mybir.EngineType.PE:
                if isinstance(i, mybir.InstLdweights):
                    k = key(i)
                    si = i.sync_info
                    clean = not (si and (si.on_wait or si.on_update))
                    if k == last and clean:
                        continue
                    last = k
                elif not isinstance(i, mybir.InstMatmult):
                    last = None
            keep.append(i)
        blk.instructions[:] = keep


def _route(expert_indices, expert_weights):
    idx = np.asarray(expert_indices).astype(np.int64)
    wts = np.asarray(expert_weights).astype(np.float32)
    n = idx.shape[0]
    cw_full = np.zeros((N_EXPERTS, n), np.float32)
    for k in range(idx.shape[1]):
        np.add.at(cw_full, (idx[:, k], np.arange(n)), wts[:, k])
    ids = [np.nonzero(cw_full[e])[0] for e in range(N_EXPERTS)]
    maxc = max(len(i) for i in ids)
    t_pad = max(1024, ((maxc + 1023) // 1024) * 1024)
    return cw_full, ids, t_pad


def _run(nc, in_maps, trace=False, trace_cores=None):
    from concourse.bass_utils import run_bass_kernel_spmd

    return run_bass_kernel_spmd(
        nc, in_maps, list(range(N_EXPERTS)), trace=trace,
        trace_cores=trace_cores,
    )


def prepare(tokens, expert_indices, expert_weights, gate_weight, up_weight,
            down_weight):
    """Host-side routing + layout. Returns (nc, in_maps, ids, t_pad)."""
    tokens = np.ascontiguousarray(np.asarray(tokens, dtype=np.float32))
    gate_weight = np.asarray(gate_weight, dtype=np.float32)
    up_weight = np.asarray(up_weight, dtype=np.float32)
    down_weight = np.asarray(down_weight, dtype=np.float32)

    cw_full, ids, t_pad = _route(expert_indices, expert_weights)
    nt = t_pad // P

    key = t_pad
    if key not in _CACHE:
        _CACHE[key] = _build(t_pad)
    nc = _CACHE[key]

    mmdt = np.dtype("bfloat16")
    in_maps = []
    for e in range(N_EXPERTS):
        ce = len(ids[e])
        xe = np.zeros((HIDDEN, t_pad), np.float32)
        xe[:, :ce] = tokens[ids[e]].T
        cwe = np.zeros((t_pad,), np.float32)
        cwe[:ce] = cw_full[e, ids[e]]
        in_maps.append({
            "xt": np.ascontiguousarray(xe.reshape(KH, P, t_pad)).astype(mmdt),
            "gw": np.ascontiguousarray(
                gate_weight[e].reshape(KH, P, NI, P).transpose(2, 1, 0, 3)
            ).reshape(NI, P, HIDDEN).astype(mmdt),
            "uw": np.ascontiguousarray(
                up_weight[e].reshape(KH, P, NI, P).transpose(2, 1, 0, 3)
            ).reshape(NI, P, HIDDEN).astype(mmdt),
            "dw": np.ascontiguousarray(down_weight[e].reshape(NI, P, HIDDEN)).astype(mmdt),
            "cw": np.ascontiguousarray(cwe.reshape(nt, P).T),
        })
    return nc, in_maps, ids, t_pad


def combine(results, ids):
    out = np.zeros((N_TOKENS, HIDDEN), np.float32)
    for e in range(N_EXPERTS):
        ce = len(ids[e])
        out[ids[e]] += results[e]["y"][:ce].astype(np.float32)
    return out


def kernel(tokens, expert_indices, expert_weights, gate_weight, up_weight,
           down_weight):
    nc, in_maps, ids, _ = prepare(tokens, expert_indices, expert_weights,
                                  gate_weight, up_weight, down_weight)
    res = _run(nc, in_maps, trace=False)
    return combine(res.results, ids)
